# revision 29
# baseline (speedup 1.0000x reference)
"""Trainium2 Bass kernel for nn_DenseContrastLoss.

Strategy (data-parallel over instances, 8 cores x 13 instances):
  - Host: gather the 96 sampled pixel-vectors per instance (32 anchor +
    32 pos + 32 neg; indices are host-known) from feats, transpose to
    channel-major, convert to fp8, ship [128, 2, 1248] per core (3
    chunks, small-first) plus fp8 weights.
  - Device (per core), software-pipelined across 3 chunks:
      L1 (fp8 DoubleRow matmul: one MM per output half) -> relu+b1 (DVE)
      L2 (fp8 DoubleRow, rhs = fp8 hs) -> sq = Square(pp+b2) (ACT, fp8)
      colsum (DoubleRow with fp8 ones) -> ln row (ACT, bf16) ->
      broadcast matmul -> rn = exp(-ln/2) (ACT, [128,cw] bf16) ->
      pn = (pp+b2)*rn (DVE scalar_tensor_tensor, straight from PSUM)
    then per-instance similarity matmuls col-tiled into one [96, 5*64]
    PSUM tile (instance n -> partition block n%3, col group n//3), and
    a max-free InfoNCE chain: term = ln(1 + sum_m' exp(an) * exp(-ap))
    (the reference's max-subtraction cancels algebraically); the
    reduction and broadcast-multiply of the chain run on GpSimd.
  - Host: validity mask from gt_mask areas, masked mean, * LOSS_WEIGHT.
"""

import sys

import numpy as np

if "/opt/trn_rl_repo" not in sys.path:
    sys.path.insert(0, "/opt/trn_rl_repo")

import ml_dtypes

import concourse.bass as bass
import concourse.tile as tile
from concourse import bacc, mybir
from concourse.bass_utils import run_bass_kernel_spmd

F32 = mybir.dt.float32
BF16 = mybir.dt.bfloat16
FP8 = mybir.dt.float8e4

TAU = 0.07
LOSS_WEIGHT = 1.2
NUM_SAMPLES = 32
C = 256
SIDE = 28
PIX = SIDE * SIDE  # 784
N_INST = 100
N_CORES = 8
NI = 13                      # instances per core (8*13 = 104 >= 100)
SAMP = 3 * NUM_SAMPLES       # 96 sampled pixels per instance
STOT = NI * SAMP             # 1248
# small-first chunking: fast pipeline fill; each chunk <= 512 (PSUM
# fp32 bank limit)
CHUNKS = [224, 512, 512]
COFF = [0, 224, 736]
NCH = len(CHUNKS)
# instances fully covered once chunk ch's pn is written
SIMS_AT = [(0, 2), (2, 7), (7, 13)]
NJ, NG = 3, 5                # sim packing: partition blocks x col groups
NWARM = 5                    # PE warm-up matmuls during the input DMA
DR = mybir.MatmulPerfMode.DoubleRow


def _build_nc():
    nc = bacc.Bacc("TRN2", target_bir_lowering=False)
    # W1 (2*256 fp8) and gather chunk 0 packed into one DMA
    wg0 = nc.declare_dram_parameter(
        "wg0", [128, 512 + 2 * CHUNKS[0] + 32], FP8, isOutput=False
    )
    gts = [None] + [
        nc.declare_dram_parameter(f"gt{ch}", [128, 2, CHUNKS[ch]], FP8,
                                  isOutput=False)
        for ch in range(1, NCH)
    ]
    w2m = nc.declare_dram_parameter("w2m", [128, 2, C], FP8, isOutput=False)
    # per-(anchor, pos) loss terms ln(1 + S_k exp(-ap)); host reduces
    loss = nc.declare_dram_parameter("loss", [96, NG * 32], BF16,
                                     isOutput=True)

    AT = mybir.ActivationFunctionType
    ALU = mybir.AluOpType
    PSUM = bass.MemorySpace.PSUM

    with tile.TileContext(nc) as tc:
        with tc.tile_pool(name="singles", bufs=1) as singles:
            # input DMAs: completion semaphores lag issue by ~2us, so
            # issue order = need order.  bcm gates the first DVE op;
            # wg0 gates the first matmul.
            WG0 = singles.tile([128, 512 + 2 * CHUNKS[0] + 32], FP8)
            nc.sync.dma_start(out=WG0[:], in_=wg0[:, :])
            gch = [None] + [singles.tile([128, 2, CHUNKS[ch]], FP8,
                                         name=f"g{ch}")
                            for ch in range(1, NCH)]
            nc.sync.dma_start(out=gch[1][:], in_=gts[1][:, :, :])
            W2 = singles.tile([128, 2, C], FP8)
            nc.scalar.dma_start(out=W2[:], in_=w2m[:, :, :])
            # ACT table set (exp/ln/square), right after the ACT-queue
            # DMA issues, well before the first activation
            nc.scalar.add_instruction(
                mybir.InstLoadActFuncSet(
                    name=nc.get_next_instruction_name(),
                    ins=[],
                    outs=[],
                    act_func_set_id=6,  # natural_log_exp_and_others
                )
            )
            # constants via gpsimd memsets, then the last gather chunks
            warm = singles.tile([128, 512], BF16, name="warm")
            nc.gpsimd.memset(warm[:], 1.0)
            nc.gpsimd.dma_start(out=gch[2][:], in_=gts[2][:, :, :])
            onesr = singles.tile([1, 128], BF16, name="onesr")
            nc.gpsimd.memset(onesr[:], 1.0)
            # dual-fp8 LDWEIGHTS needs the two k-planes 16B apart
            onesc = singles.tile([128, 2, 16], FP8, name="onesc")
            nc.gpsimd.memset(onesc[:], 1.0)

            W1 = WG0[:, :512].rearrange("p (k d) -> p k d", k=2)
            gch[0] = WG0[:, 512: 512 + 2 * CHUNKS[0]].rearrange(
                "p (k s) -> p k s", k=2
            )
            # b1/b2 ride in the tail bytes of wg0, bitcast back to f32
            BCV = WG0[:, 512 + 2 * CHUNKS[0]:].bitcast(F32)
            B1, B2 = BCV[:, 0:2], BCV[:, 2:4]

            with tc.tile_pool(name="big", bufs=1) as big:
                hs = big.tile([128, 2, STOT], FP8, name="hs")
                pn = big.tile([128, 2, STOT], BF16, name="pn")

                with (
                    tc.tile_pool(name="mmp", bufs=6, space=PSUM) as mmp,
                    tc.tile_pool(name="nsqp", bufs=1, space=PSUM) as nsqp,
                    tc.tile_pool(name="simp", bufs=1, space=PSUM) as simp,
                    tc.tile_pool(name="qsp", bufs=2) as qsp,
                ):
                    sim = simp.tile([96, 512], F32, tag="sim")

                    # PE warm-up during the input-DMA window: ramps HAM
                    # before the real matmuls arrive.  Writes land in
                    # the sim bank and are overwritten later.
                    for _ in range(NWARM):
                        nc.tensor.matmul(
                            sim[:96, :512], warm[:, :96], warm[:, :512],
                            start=True, stop=True,
                        )

                    def filler(n):
                        # keep the PE busy across a known dependency wait
                        # so the HAM clock-gate stays at full rate; write
                        # only the unused cols 320-512 of the sim bank
                        for _ in range(n):
                            nc.tensor.matmul(
                                sim[:96, 320:512], warm[:, :96],
                                warm[:, :192],
                                start=True, stop=True,
                            )

                    hp = {}
                    pp = {}
                    qs = {}
                    nsqs = {}
                    lnt = {}
                    rn = {}

                    def l1(ch):
                        cw = CHUNKS[ch]
                        hp[ch] = [mmp.tile([128, 512], F32, tag="mm",
                                           name=f"hp{ch}_{m}")
                                  for m in range(2)]
                        for m in range(2):
                            nc.tensor.matmul(
                                hp[ch][m][:, :cw],
                                W1[:, :, 128 * m: 128 * (m + 1)],
                                gch[ch][:, :, :],
                                start=True, stop=True,
                                perf_mode=DR,
                            )

                    def relu(ch):
                        # DVE: hs = max(hp + b1, 0), PSUM -> SBUF fp8
                        cw = CHUNKS[ch]
                        sl = slice(COFF[ch], COFF[ch] + cw)
                        for m in range(2):
                            nc.vector.tensor_scalar(
                                out=hs[:, m, sl], in0=hp[ch][m][:, :cw],
                                scalar1=B1[:, m: m + 1], scalar2=0.0,
                                op0=ALU.add, op1=ALU.max,
                            )

                    def l2(ch):
                        cw = CHUNKS[ch]
                        sl = slice(COFF[ch], COFF[ch] + cw)
                        pp[ch] = [mmp.tile([128, 512], F32, tag="mm",
                                           name=f"pp{ch}_{m}")
                                  for m in range(2)]
                        for m in range(2):
                            nc.tensor.matmul(
                                pp[ch][m][:, :cw],
                                W2[:, :, 128 * m: 128 * (m + 1)],
                                hs[:, :, sl],
                                start=True, stop=True,
                                perf_mode=DR,
                            )

                    def sq(ch):
                        # ACT: qs = (pp + b2)^2, fp8 out for the
                        # DoubleRow colsum
                        cw = CHUNKS[ch]
                        q = qsp.tile([128, 2, 512], FP8, tag="qs",
                                     name=f"qs{ch}")
                        for m in range(2):
                            nc.scalar.activation(
                                out=q[:, m, :cw],
                                in_=pp[ch][m][:, :cw],
                                func=AT.Square,
                                bias=B2[:, m: m + 1],
                            )
                        qs[ch] = q

                    def colsum(ch):
                        # PE: nsq = ones^T qs (fp8 DoubleRow)
                        cw = CHUNKS[ch]
                        nsq = nsqp.tile([1, 512], F32, tag="nsq")
                        nc.tensor.matmul(
                            nsq[:, :cw], onesc[:, :, :1],
                            qs[ch][:, :, :cw],
                            start=True, stop=True,
                            perf_mode=DR,
                        )
                        nsqs[ch] = nsq

                    def ln_op(ch):
                        # ACT: ln(tau*nsq) row, bf16 out
                        cw = CHUNKS[ch]
                        t = big.tile([1, 512], BF16, tag="lnt",
                                     name="lnt", bufs=2)
                        nc.scalar.activation(
                            out=t[:, :cw], in_=nsqs[ch][:, :cw], func=AT.Ln,
                            scale=float(TAU),
                        )
                        lnt[ch] = t

                    def bcast(ch):
                        # PE: broadcast ln row to 128 partitions
                        cw = CHUNKS[ch]
                        r = mmp.tile([128, 512], F32, tag="mm",
                                     name=f"rr{ch}")
                        nc.tensor.matmul(
                            r[:, :cw], onesr[:], lnt[ch][:, :cw],
                            start=True, stop=True,
                        )
                        rn[ch] = r

                    def rn_exp(ch):
                        # ACT: rn = exp(-0.5*ln(tau*nsq)), bf16
                        cw = CHUNKS[ch]
                        e = big.tile([128, 512], BF16, tag="rn",
                                     name="rn", bufs=2)
                        nc.scalar.activation(
                            out=e[:, :cw], in_=rn[ch][:, :cw], func=AT.Exp,
                            scale=-0.5,
                        )
                        rn[ch] = e

                    def pnorm(ch):
                        # DVE: pn = (pp + b2) * rn, straight from PSUM
                        cw = CHUNKS[ch]
                        sl = slice(COFF[ch], COFF[ch] + cw)
                        for m in range(2):
                            nc.vector.scalar_tensor_tensor(
                                out=pn[:, m, sl], in0=pp[ch][m][:, :cw],
                                scalar=B2[:, m: m + 1],
                                in1=rn[ch][:, :cw],
                                op0=ALU.add, op1=ALU.mult,
                            )

                    def sims(n0, n1):
                        for n in range(n0, n1):
                            a0 = SAMP * n
                            j, g = n % NJ, n // NJ
                            dst = sim[32 * j: 32 * (j + 1),
                                      64 * g: 64 * (g + 1)]
                            for k in range(2):
                                nc.tensor.matmul(
                                    dst,
                                    pn[:, k, a0: a0 + 32],
                                    pn[:, k, a0 + 32: a0 + 96],
                                    start=(k == 0),
                                    stop=(k == 1),
                                )

                    # ---- max-free InfoNCE chain ----
                    sim3 = sim[:, : NG * 64].rearrange(
                        "p (g m) -> p g m", g=NG
                    )
                    ee = big.tile([96, NG * 32], F32, name="ee")
                    s4 = big.tile([96, NG], F32, name="s4")
                    em = big.tile([96, NG * 32], F32, name="em")
                    tt = big.tile([96, NG * 32], F32, name="tt")
                    ctb = big.tile([96, NG * 32], BF16, name="ctb")

                    def chain(g0, g1):
                        gn = g1 - g0
                        c0, c1 = 32 * g0, 32 * g1
                        ee3 = ee[:, c0:c1].rearrange("p (g m) -> p g m", g=gn)
                        em3 = em[:, c0:c1].rearrange("p (g m) -> p g m", g=gn)
                        nc.scalar.activation(
                            out=ee3, in_=sim3[:, g0:g1, 32:64], func=AT.Exp,
                        )
                        nc.vector.reduce_sum(
                            out=s4[:, g0:g1], in_=ee3,
                            axis=mybir.AxisListType.X,
                        )
                        nc.scalar.activation(
                            out=em3, in_=sim3[:, g0:g1, 0:32], func=AT.Exp,
                            scale=-1.0,
                        )
                        nc.vector.tensor_mul(
                            out=tt[:, c0:c1].rearrange(
                                "p (g m) -> p g m", g=gn
                            ),
                            in0=em3,
                            in1=s4[:, g0:g1].unsqueeze(-1).broadcast_to(
                                [96, gn, 32]
                            ),
                        )
                        nc.scalar.activation(
                            out=ctb[:, c0:c1], in_=tt[:, c0:c1], func=AT.Ln,
                            bias=1.0,
                        )

                    # ---- software-pipelined issue order ----
                    l1(0); relu(0)
                    l1(1); relu(1)
                    l2(0); sq(0)
                    l1(2); relu(2)
                    colsum(0); ln_op(0)
                    l2(1); sq(1)
                    bcast(0); rn_exp(0); pnorm(0)
                    filler(2)
                    sims(*SIMS_AT[0])
                    colsum(1); ln_op(1)
                    l2(2); sq(2)
                    bcast(1); rn_exp(1); pnorm(1)
                    filler(2)
                    sims(*SIMS_AT[1])
                    chain(0, 2)          # instances 0-5
                    nc.sync.dma_start(out=loss[:, :64], in_=ctb[:, :64])
                    colsum(2); ln_op(2); bcast(2); rn_exp(2); pnorm(2)
                    sims(*SIMS_AT[2])
                    chain(2, NG)         # instances 6-12
                    # garbage in the two unused slots (g=4, j=1,2) stays
                    # in its blocks; the host only reads valid ones
                    nc.scalar.dma_start(out=loss[:, 64:], in_=ctb[:, 64:])

    nc.compile()
    return nc


_NC_CACHE = None


def _get_nc():
    global _NC_CACHE
    if _NC_CACHE is None:
        _NC_CACHE = _build_nc()
    return _NC_CACHE


def _host_prep(feats, w1, b1, w2, b2, anchor_inds, pos_inds, neg_inds):
    """Build the 8 per-core input maps."""
    n = feats.shape[0]
    ntot = N_CORES * NI
    ff = np.asarray(feats, dtype=np.float32).reshape(n, C, PIX)

    def flat(inds):
        inds = np.asarray(inds)
        f = inds[..., 0].astype(np.int64) * SIDE + inds[..., 1].astype(np.int64)
        if ntot > n:
            f = np.concatenate(
                [f, np.broadcast_to(f[0], (ntot - n,) + f.shape[1:])], axis=0
            )
        return f  # [ntot, 32]

    af, pf, nf = flat(anchor_inds), flat(pos_inds), flat(neg_inds)
    samp = np.concatenate([af, pf, nf], axis=1)  # [ntot, 96]
    idx = np.minimum(np.arange(ntot), n - 1)
    g = np.take_along_axis(ff[idx], samp[:, None, :], axis=2)
    # per-core channel-major [C, 1248] -> fp8 chunks [128, 2, cw]
    g = g.reshape(N_CORES, NI, C, SAMP)
    g = np.transpose(g, (0, 2, 1, 3)).reshape(N_CORES, C, STOT)
    g = g.astype(ml_dtypes.float8_e4m3)
    g = g.reshape(N_CORES, 2, 128, STOT)  # c = k*128 + p
    g = np.transpose(g, (0, 2, 1, 3))     # [cores, 128, 2, STOT]
    gchunks = [
        np.ascontiguousarray(g[:, :, :, COFF[ch]: COFF[ch] + CHUNKS[ch]])
        for ch in range(NCH)
    ]

    def wprep(w):
        wt = np.asarray(w, dtype=np.float32).T  # [c, d]
        wt = wt.reshape(2, 128, C)              # [k, p, d]
        return np.ascontiguousarray(
            np.transpose(wt, (1, 0, 2)).astype(ml_dtypes.float8_e4m3)
        )

    w1p = wprep(w1)
    w2p = wprep(w2)
    bcm = np.zeros((128, 8), dtype=np.float32)
    bcm[:, 0:2] = np.asarray(b1, dtype=np.float32).reshape(2, 128).T
    bcm[:, 2:4] = np.asarray(b2, dtype=np.float32).reshape(2, 128).T
    bcm8 = np.ascontiguousarray(bcm).view(ml_dtypes.float8_e4m3)

    w1flat = np.ascontiguousarray(w1p.reshape(128, 512))
    in_maps = []
    for c in range(N_CORES):
        wg0 = np.concatenate(
            [w1flat, gchunks[0][c].reshape(128, 2 * CHUNKS[0]), bcm8],
            axis=1,
        )
        m = {"wg0": np.ascontiguousarray(wg0), "w2m": w2p}
        for ch in range(1, NCH):
            m[f"gt{ch}"] = gchunks[ch][c]
        in_maps.append(m)
    return in_maps


def _finalize(loss_per, gt_mask):
    gt = np.asarray(gt_mask)
    area = gt.reshape(gt.shape[0], -1).sum(axis=1)
    valid = (area > NUM_SAMPLES) & (area < PIX - NUM_SAMPLES)
    n_valid = np.float32(valid.sum())
    if n_valid > 0:
        total = np.float32(np.where(valid, loss_per, 0.0).astype(np.float32).sum())
        out = total / max(n_valid, np.float32(1.0))
    else:
        out = np.float32(0.0)
    return np.float32(out * np.float32(LOSS_WEIGHT))


def kernel(feats, w1, b1, w2, b2, gt_mask, anchor_inds, pos_inds, neg_inds,
           _results_hook=None):
    nc = _get_nc()
    in_maps = _host_prep(feats, w1, b1, w2, b2, anchor_inds, pos_inds, neg_inds)
    res = run_bass_kernel_spmd(nc, in_maps, list(range(N_CORES)))
    if _results_hook is not None:
        _results_hook(res)
    parts = []
    for c in range(N_CORES):
        ctb = np.asarray(res.results[c]["loss"], dtype=np.float32)
        for n in range(NI):
            j, g = n % NJ, n // NJ
            blk = ctb[32 * j: 32 * (j + 1), 32 * g: 32 * (g + 1)]
            parts.append(blk.sum(dtype=np.float32))
    loss_per = np.array(parts, dtype=np.float32)[: N_INST] / np.float32(
        NUM_SAMPLES * NUM_SAMPLES
    )
    return _finalize(loss_per, gt_mask)


# revision 30
# speedup vs baseline: 1.0057x; 1.0057x over previous
"""Trainium2 Bass kernel for nn_DenseContrastLoss.

Strategy (data-parallel over instances, 8 cores x 13 instances):
  - Host: gather the 96 sampled pixel-vectors per instance (32 anchor +
    32 pos + 32 neg; indices are host-known) from feats, transpose to
    channel-major, convert to fp8, ship [128, 2, 1248] per core (3
    chunks, small-first) plus fp8 weights.
  - Device (per core), software-pipelined across 3 chunks:
      L1 (fp8 DoubleRow matmul: one MM per output half) -> relu+b1 (DVE)
      L2 (fp8 DoubleRow, rhs = fp8 hs) -> sq = Square(pp+b2) (ACT, fp8)
      colsum (DoubleRow with fp8 ones) -> ln row (ACT, bf16) ->
      broadcast matmul -> rn = exp(-ln/2) (ACT, [128,cw] bf16) ->
      pn = (pp+b2)*rn (DVE scalar_tensor_tensor, straight from PSUM)
    then per-instance similarity matmuls col-tiled into one [96, 5*64]
    PSUM tile (instance n -> partition block n%3, col group n//3), and
    a max-free InfoNCE chain: term = ln(1 + sum_m' exp(an) * exp(-ap))
    (the reference's max-subtraction cancels algebraically); the
    reduction and broadcast-multiply of the chain run on GpSimd.
  - Host: validity mask from gt_mask areas, masked mean, * LOSS_WEIGHT.
"""

import sys

import numpy as np

if "/opt/trn_rl_repo" not in sys.path:
    sys.path.insert(0, "/opt/trn_rl_repo")

import ml_dtypes

import concourse.bass as bass
import concourse.tile as tile
from concourse import bacc, mybir
from concourse.bass_utils import run_bass_kernel_spmd

F32 = mybir.dt.float32
BF16 = mybir.dt.bfloat16
FP8 = mybir.dt.float8e4

TAU = 0.07
LOSS_WEIGHT = 1.2
NUM_SAMPLES = 32
C = 256
SIDE = 28
PIX = SIDE * SIDE  # 784
N_INST = 100
N_CORES = 8
NI = 13                      # instances per core (8*13 = 104 >= 100)
SAMP = 3 * NUM_SAMPLES       # 96 sampled pixels per instance
STOT = NI * SAMP             # 1248
# small-first chunking: fast pipeline fill; each chunk <= 512 (PSUM
# fp32 bank limit)
CHUNKS = [224, 512, 512]
COFF = [0, 224, 736]
NCH = len(CHUNKS)
# instances fully covered once chunk ch's pn is written
SIMS_AT = [(0, 2), (2, 7), (7, 13)]
NJ, NG = 3, 5                # sim packing: partition blocks x col groups
NWARM = 5                    # PE warm-up matmuls during the input DMA
DR = mybir.MatmulPerfMode.DoubleRow


def _build_nc():
    nc = bacc.Bacc("TRN2", target_bir_lowering=False)
    # W1 (2*256 fp8) and gather chunk 0 packed into one DMA
    wg0 = nc.declare_dram_parameter(
        "wg0", [128, 512 + 2 * CHUNKS[0] + 32], FP8, isOutput=False
    )
    gts = [None] + [
        nc.declare_dram_parameter(f"gt{ch}", [128, 2, CHUNKS[ch]], FP8,
                                  isOutput=False)
        for ch in range(1, NCH)
    ]
    w2m = nc.declare_dram_parameter("w2m", [128, 2, C], FP8, isOutput=False)
    # per-(anchor, pos) loss terms ln(1 + S_k exp(-ap)); host reduces
    loss = nc.declare_dram_parameter("loss", [96, NG * 32], BF16,
                                     isOutput=True)

    AT = mybir.ActivationFunctionType
    ALU = mybir.AluOpType
    PSUM = bass.MemorySpace.PSUM

    with tile.TileContext(nc) as tc:
        with tc.tile_pool(name="singles", bufs=1) as singles:
            # input DMAs: completion semaphores lag issue by ~2us, so
            # issue order = need order.  bcm gates the first DVE op;
            # wg0 gates the first matmul.
            WG0 = singles.tile([128, 512 + 2 * CHUNKS[0] + 32], FP8)
            nc.sync.dma_start(out=WG0[:], in_=wg0[:, :])
            gch = [None] + [singles.tile([128, 2, CHUNKS[ch]], FP8,
                                         name=f"g{ch}")
                            for ch in range(1, NCH)]
            nc.sync.dma_start(out=gch[1][:], in_=gts[1][:, :, :])
            W2 = singles.tile([128, 2, C], FP8)
            nc.scalar.dma_start(out=W2[:], in_=w2m[:, :, :])
            # ACT table set (exp/ln/square), right after the ACT-queue
            # DMA issues, well before the first activation
            nc.scalar.add_instruction(
                mybir.InstLoadActFuncSet(
                    name=nc.get_next_instruction_name(),
                    ins=[],
                    outs=[],
                    act_func_set_id=6,  # natural_log_exp_and_others
                )
            )
            # constants via gpsimd memsets, then the last gather chunks
            warm = singles.tile([128, 512], BF16, name="warm")
            nc.gpsimd.memset(warm[:], 1.0)
            nc.gpsimd.dma_start(out=gch[2][:], in_=gts[2][:, :, :])
            onesr = singles.tile([1, 128], BF16, name="onesr")
            nc.gpsimd.memset(onesr[:], 1.0)
            # dual-fp8 LDWEIGHTS needs the two k-planes 16B apart
            onesc = singles.tile([128, 2, 16], FP8, name="onesc")
            nc.gpsimd.memset(onesc[:], 1.0)

            W1 = WG0[:, :512].rearrange("p (k d) -> p k d", k=2)
            gch[0] = WG0[:, 512: 512 + 2 * CHUNKS[0]].rearrange(
                "p (k s) -> p k s", k=2
            )
            # b1/b2 ride in the tail bytes of wg0, bitcast back to f32
            BCV = WG0[:, 512 + 2 * CHUNKS[0]:].bitcast(F32)
            B1, B2 = BCV[:, 0:2], BCV[:, 2:4]

            with tc.tile_pool(name="big", bufs=1) as big:
                hs = big.tile([128, 2, STOT], FP8, name="hs")
                pn = big.tile([128, 2, STOT], BF16, name="pn")

                with (
                    tc.tile_pool(name="mmp", bufs=6, space=PSUM) as mmp,
                    tc.tile_pool(name="nsqp", bufs=1, space=PSUM) as nsqp,
                    tc.tile_pool(name="simp", bufs=1, space=PSUM) as simp,
                    tc.tile_pool(name="qsp", bufs=2) as qsp,
                ):
                    sim = simp.tile([96, 512], F32, tag="sim")

                    # PE warm-up during the input-DMA window: ramps HAM
                    # before the real matmuls arrive.  Writes land in
                    # the sim bank and are overwritten later.
                    for _ in range(NWARM):
                        nc.tensor.matmul(
                            sim[:96, :512], warm[:, :96], warm[:, :512],
                            start=True, stop=True,
                        )

                    def filler(n):
                        # keep the PE busy across a known dependency wait
                        # so the HAM clock-gate stays at full rate; write
                        # only the unused cols 320-512 of the sim bank
                        for _ in range(n):
                            nc.tensor.matmul(
                                sim[:96, 320:512], warm[:, :96],
                                warm[:, :192],
                                start=True, stop=True,
                            )

                    hp = {}
                    pp = {}
                    qs = {}
                    nsqs = {}
                    lnt = {}
                    rn = {}

                    def l1(ch):
                        cw = CHUNKS[ch]
                        hp[ch] = [mmp.tile([128, 512], F32, tag="mm",
                                           name=f"hp{ch}_{m}")
                                  for m in range(2)]
                        for m in range(2):
                            nc.tensor.matmul(
                                hp[ch][m][:, :cw],
                                W1[:, :, 128 * m: 128 * (m + 1)],
                                gch[ch][:, :, :],
                                start=True, stop=True,
                                perf_mode=DR,
                            )

                    def relu(ch):
                        # DVE: hs = max(hp + b1, 0), PSUM -> SBUF fp8
                        cw = CHUNKS[ch]
                        sl = slice(COFF[ch], COFF[ch] + cw)
                        for m in range(2):
                            nc.vector.tensor_scalar(
                                out=hs[:, m, sl], in0=hp[ch][m][:, :cw],
                                scalar1=B1[:, m: m + 1], scalar2=0.0,
                                op0=ALU.add, op1=ALU.max,
                            )

                    def l2(ch):
                        cw = CHUNKS[ch]
                        sl = slice(COFF[ch], COFF[ch] + cw)
                        pp[ch] = [mmp.tile([128, 512], F32, tag="mm",
                                           name=f"pp{ch}_{m}")
                                  for m in range(2)]
                        for m in range(2):
                            nc.tensor.matmul(
                                pp[ch][m][:, :cw],
                                W2[:, :, 128 * m: 128 * (m + 1)],
                                hs[:, :, sl],
                                start=True, stop=True,
                                perf_mode=DR,
                            )

                    def sq(ch):
                        # ACT: qs = (pp + b2)^2, fp8 out for the
                        # DoubleRow colsum
                        cw = CHUNKS[ch]
                        q = qsp.tile([128, 2, 512], FP8, tag="qs",
                                     name=f"qs{ch}")
                        for m in range(2):
                            nc.scalar.activation(
                                out=q[:, m, :cw],
                                in_=pp[ch][m][:, :cw],
                                func=AT.Square,
                                bias=B2[:, m: m + 1],
                            )
                        qs[ch] = q

                    def colsum(ch):
                        # PE: nsq = ones^T qs (fp8 DoubleRow)
                        cw = CHUNKS[ch]
                        nsq = nsqp.tile([1, 512], F32, tag="nsq")
                        nc.tensor.matmul(
                            nsq[:, :cw], onesc[:, :, :1],
                            qs[ch][:, :, :cw],
                            start=True, stop=True,
                            perf_mode=DR,
                        )
                        nsqs[ch] = nsq

                    def ln_op(ch):
                        # ACT: ln(tau*nsq) row, bf16 out
                        cw = CHUNKS[ch]
                        t = big.tile([1, 512], BF16, tag="lnt",
                                     name="lnt", bufs=2)
                        nc.scalar.activation(
                            out=t[:, :cw], in_=nsqs[ch][:, :cw], func=AT.Ln,
                            scale=float(TAU),
                        )
                        lnt[ch] = t

                    def bcast(ch):
                        # PE: broadcast ln row to 128 partitions
                        cw = CHUNKS[ch]
                        r = mmp.tile([128, 512], F32, tag="mm",
                                     name=f"rr{ch}")
                        nc.tensor.matmul(
                            r[:, :cw], onesr[:], lnt[ch][:, :cw],
                            start=True, stop=True,
                        )
                        rn[ch] = r

                    def rn_exp(ch):
                        # ACT: rn = exp(-0.5*ln(tau*nsq)), bf16
                        cw = CHUNKS[ch]
                        e = big.tile([128, 512], BF16, tag="rn",
                                     name="rn", bufs=2)
                        nc.scalar.activation(
                            out=e[:, :cw], in_=rn[ch][:, :cw], func=AT.Exp,
                            scale=-0.5,
                        )
                        rn[ch] = e

                    def pnorm(ch):
                        # DVE: pn = (pp + b2) * rn, straight from PSUM
                        cw = CHUNKS[ch]
                        sl = slice(COFF[ch], COFF[ch] + cw)
                        for m in range(2):
                            nc.vector.scalar_tensor_tensor(
                                out=pn[:, m, sl], in0=pp[ch][m][:, :cw],
                                scalar=B2[:, m: m + 1],
                                in1=rn[ch][:, :cw],
                                op0=ALU.add, op1=ALU.mult,
                            )

                    def sims(n0, n1):
                        for n in range(n0, n1):
                            a0 = SAMP * n
                            j, g = n % NJ, n // NJ
                            dst = sim[32 * j: 32 * (j + 1),
                                      64 * g: 64 * (g + 1)]
                            for k in range(2):
                                nc.tensor.matmul(
                                    dst,
                                    pn[:, k, a0: a0 + 32],
                                    pn[:, k, a0 + 32: a0 + 96],
                                    start=(k == 0),
                                    stop=(k == 1),
                                )

                    # ---- max-free InfoNCE chain ----
                    sim3 = sim[:, : NG * 64].rearrange(
                        "p (g m) -> p g m", g=NG
                    )
                    ee = big.tile([96, NG * 32], F32, name="ee")
                    s4 = big.tile([96, NG], F32, name="s4")
                    em = big.tile([96, NG * 32], F32, name="em")
                    tt = big.tile([96, NG * 32], F32, name="tt")
                    ctb = big.tile([96, NG * 32], BF16, name="ctb")

                    def chain(g0, g1):
                        gn = g1 - g0
                        c0, c1 = 32 * g0, 32 * g1
                        ee3 = ee[:, c0:c1].rearrange("p (g m) -> p g m", g=gn)
                        em3 = em[:, c0:c1].rearrange("p (g m) -> p g m", g=gn)
                        nc.scalar.activation(
                            out=ee3, in_=sim3[:, g0:g1, 32:64], func=AT.Exp,
                        )
                        nc.vector.reduce_sum(
                            out=s4[:, g0:g1], in_=ee3,
                            axis=mybir.AxisListType.X,
                        )
                        nc.scalar.activation(
                            out=em3, in_=sim3[:, g0:g1, 0:32], func=AT.Exp,
                            scale=-1.0,
                        )
                        nc.vector.tensor_mul(
                            out=tt[:, c0:c1].rearrange(
                                "p (g m) -> p g m", g=gn
                            ),
                            in0=em3,
                            in1=s4[:, g0:g1].unsqueeze(-1).broadcast_to(
                                [96, gn, 32]
                            ),
                        )
                        nc.scalar.activation(
                            out=ctb[:, c0:c1], in_=tt[:, c0:c1], func=AT.Ln,
                            bias=1.0,
                        )

                    # ---- software-pipelined issue order ----
                    l1(0); relu(0)
                    l1(1); relu(1)
                    l2(0); sq(0)
                    l1(2); relu(2)
                    colsum(0); ln_op(0)
                    l2(1); sq(1)
                    bcast(0); rn_exp(0); pnorm(0)
                    filler(2)
                    sims(*SIMS_AT[0])
                    colsum(1); ln_op(1)
                    l2(2); sq(2)
                    bcast(1); rn_exp(1); pnorm(1)
                    sims(*SIMS_AT[1])
                    chain(0, 2)          # instances 0-5
                    nc.sync.dma_start(out=loss[:, :64], in_=ctb[:, :64])
                    colsum(2); ln_op(2); bcast(2); rn_exp(2); pnorm(2)
                    sims(*SIMS_AT[2])
                    chain(2, NG)         # instances 6-12
                    # garbage in the two unused slots (g=4, j=1,2) stays
                    # in its blocks; the host only reads valid ones
                    nc.scalar.dma_start(out=loss[:, 64:], in_=ctb[:, 64:])

    nc.compile()
    return nc


_NC_CACHE = None


def _get_nc():
    global _NC_CACHE
    if _NC_CACHE is None:
        _NC_CACHE = _build_nc()
    return _NC_CACHE


def _host_prep(feats, w1, b1, w2, b2, anchor_inds, pos_inds, neg_inds):
    """Build the 8 per-core input maps."""
    n = feats.shape[0]
    ntot = N_CORES * NI
    ff = np.asarray(feats, dtype=np.float32).reshape(n, C, PIX)

    def flat(inds):
        inds = np.asarray(inds)
        f = inds[..., 0].astype(np.int64) * SIDE + inds[..., 1].astype(np.int64)
        if ntot > n:
            f = np.concatenate(
                [f, np.broadcast_to(f[0], (ntot - n,) + f.shape[1:])], axis=0
            )
        return f  # [ntot, 32]

    af, pf, nf = flat(anchor_inds), flat(pos_inds), flat(neg_inds)
    samp = np.concatenate([af, pf, nf], axis=1)  # [ntot, 96]
    idx = np.minimum(np.arange(ntot), n - 1)
    g = np.take_along_axis(ff[idx], samp[:, None, :], axis=2)
    # per-core channel-major [C, 1248] -> fp8 chunks [128, 2, cw]
    g = g.reshape(N_CORES, NI, C, SAMP)
    g = np.transpose(g, (0, 2, 1, 3)).reshape(N_CORES, C, STOT)
    g = g.astype(ml_dtypes.float8_e4m3)
    g = g.reshape(N_CORES, 2, 128, STOT)  # c = k*128 + p
    g = np.transpose(g, (0, 2, 1, 3))     # [cores, 128, 2, STOT]
    gchunks = [
        np.ascontiguousarray(g[:, :, :, COFF[ch]: COFF[ch] + CHUNKS[ch]])
        for ch in range(NCH)
    ]

    def wprep(w):
        wt = np.asarray(w, dtype=np.float32).T  # [c, d]
        wt = wt.reshape(2, 128, C)              # [k, p, d]
        return np.ascontiguousarray(
            np.transpose(wt, (1, 0, 2)).astype(ml_dtypes.float8_e4m3)
        )

    w1p = wprep(w1)
    w2p = wprep(w2)
    bcm = np.zeros((128, 8), dtype=np.float32)
    bcm[:, 0:2] = np.asarray(b1, dtype=np.float32).reshape(2, 128).T
    bcm[:, 2:4] = np.asarray(b2, dtype=np.float32).reshape(2, 128).T
    bcm8 = np.ascontiguousarray(bcm).view(ml_dtypes.float8_e4m3)

    w1flat = np.ascontiguousarray(w1p.reshape(128, 512))
    in_maps = []
    for c in range(N_CORES):
        wg0 = np.concatenate(
            [w1flat, gchunks[0][c].reshape(128, 2 * CHUNKS[0]), bcm8],
            axis=1,
        )
        m = {"wg0": np.ascontiguousarray(wg0), "w2m": w2p}
        for ch in range(1, NCH):
            m[f"gt{ch}"] = gchunks[ch][c]
        in_maps.append(m)
    return in_maps


def _finalize(loss_per, gt_mask):
    gt = np.asarray(gt_mask)
    area = gt.reshape(gt.shape[0], -1).sum(axis=1)
    valid = (area > NUM_SAMPLES) & (area < PIX - NUM_SAMPLES)
    n_valid = np.float32(valid.sum())
    if n_valid > 0:
        total = np.float32(np.where(valid, loss_per, 0.0).astype(np.float32).sum())
        out = total / max(n_valid, np.float32(1.0))
    else:
        out = np.float32(0.0)
    return np.float32(out * np.float32(LOSS_WEIGHT))


def kernel(feats, w1, b1, w2, b2, gt_mask, anchor_inds, pos_inds, neg_inds,
           _results_hook=None):
    nc = _get_nc()
    in_maps = _host_prep(feats, w1, b1, w2, b2, anchor_inds, pos_inds, neg_inds)
    res = run_bass_kernel_spmd(nc, in_maps, list(range(N_CORES)))
    if _results_hook is not None:
        _results_hook(res)
    parts = []
    for c in range(N_CORES):
        ctb = np.asarray(res.results[c]["loss"], dtype=np.float32)
        for n in range(NI):
            j, g = n % NJ, n // NJ
            blk = ctb[32 * j: 32 * (j + 1), 32 * g: 32 * (g + 1)]
            parts.append(blk.sum(dtype=np.float32))
    loss_per = np.array(parts, dtype=np.float32)[: N_INST] / np.float32(
        NUM_SAMPLES * NUM_SAMPLES
    )
    return _finalize(loss_per, gt_mask)
